# revision 2
# baseline (speedup 1.0000x reference)
"""CBOW negative-sampling loss on 8 Trainium2 NeuronCores.

Strategy: data-parallel over the batch. Each core processes B/8 = 2048
examples against fully-replicated fp16 embedding tables in its HBM.

The kernel is bound by SWDGE indirect-DMA issue rate: every gather op
(128 rows, one per partition) costs ~1.44 us of serialized Pool-engine
descriptor generation, and 2048 examples need 19 rows each -> 304 ops.
Measured: the per-op cost is ~all fixed (independent of row bytes or
queue count), so the design minimizes op count (it is already at the
128-rows/op hardware max) and keeps the Pool stream stall-free:

  - Tables are cast to fp16 on the host (max rel err ~2e-3 vs the 2e-2
    gate). This halves SBUF so ALL gathered rows (9.7 MB) fit in one
    arena with no ring reuse -> no Pool-side waits between gathers.
  - All 304 gathers are issued back-to-back; DVE/ACT compute trails
    behind per-chunk, fully hidden under the gather stream.
  - Index regions are packed on the host so every DVE op is a flat
    2-dim AP: ctx rows land position-major (folds of contiguous
    halves), neg rows k-major (flat muls against v).

Per chunk of CT tiles: v = fp16 fold of 8 ctx slabs; scores = fp16 mul
+ f32 X-reduce; loss = -ln sig(+-s/8) via ACT sigmoid(scale)+ln, then a
negated X-reduce over the 11 slots.
"""
import numpy as np

import concourse.bacc as bacc
import concourse.bass as bass
import concourse.mybir as mybir
from concourse.bass import IndirectOffsetOnAxis
from concourse.bass_utils import run_bass_kernel_spmd
from concourse.tile import TileContext

P = 128
VOCAB, D = 100000, 128
B, CTX, K = 16384, 8, 10
NCORES = 8
B_SHARD = B // NCORES          # 2048
NTILES = B_SHARD // P          # 16
CT = 2                         # tiles per compute chunk
F32 = mybir.dt.float32
F16 = mybir.dt.float16
I32 = mybir.dt.int32

_QN = [0]


def _q(inst):
    """Round-robin SWDGE queue assignment for indirect DMAs."""
    qi = _QN[0] % 4
    _QN[0] += 1
    if qi:
        inst.ins.queue = f"qPoolDynamic{qi}"
    return inst


def build(vocab=VOCAB, ntiles=NTILES, ct=CT, loop_n=None) -> bass.Bass:
    """loop_n: if set, wrap the whole body in a device-side repeat loop
    (benchmarking only — output is idempotent)."""
    from contextlib import nullcontext

    nchunk = ntiles // ct
    off_tgt = ntiles * CTX
    off_neg = ntiles * (CTX + 1)
    nidx = ntiles * (CTX + 1 + K)

    nc = bacc.Bacc("TRN2", target_bir_lowering=False, debug=False,
                   num_devices=NCORES, num_swdge_queues=4)
    in_embed = nc.dram_tensor("in_embed", [vocab, D], F16, kind="ExternalInput")
    out_embed = nc.dram_tensor("out_embed", [vocab, D], F16, kind="ExternalInput")
    idx = nc.dram_tensor("idx", [P, nidx], I32, kind="ExternalInput")
    loss = nc.dram_tensor("loss", [P, ntiles], F32, kind="ExternalOutput")

    # arena region widths (in elements per partition), per chunk
    w_ctx = CTX * ct * D
    w_pos = ct * D
    w_neg = K * ct * D
    w_chunk = w_ctx + w_pos + w_neg

    with TileContext(nc) as tc:
        with (
            tc.tile_pool(name="const", bufs=1) as cpool,
            tc.tile_pool(name="arena", bufs=1) as apool,
            tc.tile_pool(name="work", bufs=2) as work,
        ):
            idx_t = cpool.tile([P, nidx], I32)
            nc.sync.dma_start(out=idx_t[:], in_=idx[:])

            big = apool.tile([P, nchunk * w_chunk], F16)

            loop_cm = tc.For_i(0, loop_n, 1) if loop_n else nullcontext()
            with loop_cm:
                # ---- issue ALL gathers first: no ring reuse, no waits ----
                views = []
                for c in range(nchunk):
                    base = c * w_chunk
                    ctx_g = big[:, base:base + w_ctx]
                    pos_g = big[:, base + w_ctx:base + w_ctx + w_pos]
                    neg_g = big[:, base + w_ctx + w_pos:base + w_chunk]
                    views.append((ctx_g, pos_g, neg_g))
                    for j in range(ct * CTX):
                        _q(nc.gpsimd.indirect_dma_start(
                            out=ctx_g[:, j * D:(j + 1) * D], out_offset=None,
                            in_=in_embed[:],
                            in_offset=IndirectOffsetOnAxis(
                                ap=idx_t[:, c * ct * CTX + j:
                                         c * ct * CTX + j + 1], axis=0)))
                    for j in range(ct):
                        _q(nc.gpsimd.indirect_dma_start(
                            out=pos_g[:, j * D:(j + 1) * D], out_offset=None,
                            in_=out_embed[:],
                            in_offset=IndirectOffsetOnAxis(
                                ap=idx_t[:, off_tgt + c * ct + j:
                                         off_tgt + c * ct + j + 1], axis=0)))
                    for j in range(ct * K):
                        _q(nc.gpsimd.indirect_dma_start(
                            out=neg_g[:, j * D:(j + 1) * D], out_offset=None,
                            in_=out_embed[:],
                            in_offset=IndirectOffsetOnAxis(
                                ap=idx_t[:, off_neg + c * ct * K + j:
                                         off_neg + c * ct * K + j + 1],
                                axis=0)))

                # ---- compute trails the gather stream, chunk by chunk ----
                for c in range(nchunk):
                    ctx_g, pos_g, neg_g = views[c]
                    w = ct * D
                    # v_sum: fold contiguous halves (position-major layout)
                    for half in (4, 2, 1):
                        nc.vector.tensor_add(
                            out=ctx_g[:, 0:half * w],
                            in0=ctx_g[:, 0:half * w],
                            in1=ctx_g[:, half * w:2 * half * w])
                    v = ctx_g[:, 0:w]  # [P, ct*D] fp16, sum of 8 ctx rows

                    # pos scores
                    nc.vector.tensor_mul(out=pos_g[:], in0=pos_g[:], in1=v)
                    s_pos = work.tile([P, ct], F32, tag="spos")
                    nc.vector.reduce_sum(
                        out=s_pos[:],
                        in_=pos_g[:].rearrange("p (t d) -> p t d", d=D),
                        axis=mybir.AxisListType.X)

                    # neg scores: one flat mul per k (k-major layout)
                    for k in range(K):
                        nc.vector.tensor_mul(
                            out=neg_g[:, k * w:(k + 1) * w],
                            in0=neg_g[:, k * w:(k + 1) * w], in1=v)
                    s_neg = work.tile([P, K * ct], F32, tag="sneg")
                    nc.vector.reduce_sum(
                        out=s_neg[:],
                        in_=neg_g[:].rearrange("p (k d) -> p k d", d=D),
                        axis=mybir.AxisListType.X)

                    # sig_all layout [P, (1+K), ct]: pos slab then k slabs
                    sig_all = work.tile([P, (K + 1) * ct], F32, tag="sig")
                    nc.scalar.activation(
                        out=sig_all[:, 0:ct], in_=s_pos[:],
                        func=mybir.ActivationFunctionType.Sigmoid,
                        scale=1.0 / CTX)
                    nc.scalar.activation(
                        out=sig_all[:, ct:(K + 1) * ct], in_=s_neg[:],
                        func=mybir.ActivationFunctionType.Sigmoid,
                        scale=-1.0 / CTX)
                    nc.scalar.activation(
                        out=sig_all[:], in_=sig_all[:],
                        func=mybir.ActivationFunctionType.Ln)

                    # loss[p, t] = -sum_j sig_all[p, j, t]
                    loss_t = work.tile([P, ct], F32, tag="losst")
                    nc.vector.tensor_reduce(
                        out=loss_t[:],
                        in_=sig_all[:].rearrange("p (j t) -> p j t", t=ct)
                            .transpose([0, 2, 1]),
                        op=mybir.AluOpType.add,
                        axis=mybir.AxisListType.X, negate=True)
                    nc.sync.dma_start(
                        out=loss[:, c * ct:(c + 1) * ct], in_=loss_t[:])
    nc.finalize()
    return nc


def _pack_core_idx(context, target, negatives, ntiles=NTILES, ct=CT):
    """[B_shard,*] int arrays -> [P, nidx] i32.

    Example (c*ct + t)*P + p lives at partition p, chunk c, tile-slot t.
    ctx region per chunk is position-major [CTX, ct]; tgt is [ct];
    neg region per chunk is k-major [K, ct].
    """
    nchunk = ntiles // ct
    ctx_idx = (context.reshape(nchunk, ct, P, CTX)
               .transpose(2, 0, 3, 1).reshape(P, ntiles * CTX))
    tgt_idx = target.reshape(nchunk, ct, P).transpose(2, 0, 1).reshape(P, ntiles)
    neg_idx = (negatives.reshape(nchunk, ct, P, K)
               .transpose(2, 0, 3, 1).reshape(P, ntiles * K))
    return np.ascontiguousarray(
        np.concatenate([ctx_idx, tgt_idx, neg_idx], axis=1).astype(np.int32))


def _run(inputs, trace=False, loop_n=None):
    in_embed = np.asarray(inputs["in_embed"], dtype=np.float32).astype(np.float16)
    out_embed = np.asarray(inputs["out_embed"], dtype=np.float32).astype(np.float16)
    context = np.asarray(inputs["context"]).astype(np.int32)
    target = np.asarray(inputs["target"]).astype(np.int32)
    negatives = np.asarray(inputs["negatives"]).astype(np.int32)
    assert context.shape == (B, CTX) and target.shape == (B,) and negatives.shape == (B, K)

    nc = build(loop_n=loop_n)
    in_maps = []
    for i in range(NCORES):
        sl = slice(i * B_SHARD, (i + 1) * B_SHARD)
        in_maps.append({
            "in_embed": in_embed,
            "out_embed": out_embed,
            "idx": _pack_core_idx(context[sl], target[sl], negatives[sl]),
        })
    res = run_bass_kernel_spmd(nc, in_maps, core_ids=list(range(NCORES)),
                               trace=trace)
    loss = np.concatenate(
        [res.results[i]["loss"].T.reshape(-1) for i in range(NCORES)])
    return loss.astype(np.float32), res


def kernel(**inputs) -> np.ndarray:
    return _run(inputs, trace=False)[0]
